# revision 17
# baseline (speedup 1.0000x reference)
"""Trainium2 Bass kernel for EnhancedMultiHeadAttention (LoRA MHA + residual + LayerNorm).

Contract: kernel(**inputs) takes the FULL unsharded inputs (as produced by
setup_inputs()) and returns the full outputs (normed, attn.mean(axis=1)).

Sharding: 8 cores = 4 batches x 2 query-halves. Each core computes K/V for the
whole sequence of its batch and attention + output-projection + LayerNorm for
its 512 query rows.

Key optimizations over the straightforward version:
  - Algebraic folds (host side, exact): k_b shifts every score row by a
    constant -> softmax-invariant -> dropped entirely. v_b contributes
    v_b @ W_o^eff to every output row (probs sum to 1) -> folded into o_b.
    LoRA folded into dense weights. 1/sqrt(d_k) folded into W_q.
  - V and O projections run in fp8(e4m3) with DoubleRow perf mode (2 fp8
    weights per PE cell, 256-deep contraction per pass). Weights are scaled
    x16 on the host to clear the fp8 subnormal range; all scales cancel
    exactly through the pipeline (exp bias ln16, 1/16 on the reciprocal,
    x16 on the residual, eps*256 in LayerNorm). Q/K stay bf16 because
    attn_mean error depends only on them.
  - Per-pair software pipelining: the PE stream interleaves K/Q projections
    of pair i, PV of pair i-1 and scores of pair i so the tensor engine
    never idles (keeps the HAM clock-gate at 2.4GHz) while ScalarE runs exp
    and VectorE runs the attention-mean accumulation.
  - 8 attention-mean accumulators (one per head pair, summed on the host
    during unsharding): only 3 wide DVE ops per pair and a 1-deep bf16
    accumulation chain.
  - The per-query 1/denominator row is broadcast across partitions by the
    (otherwise idle) GpSimd engine (partition_broadcast) instead of a
    PE-matmul + ScalarE copy.
"""

import sys
import numpy as np

_REPO = "/opt/trn_rl_repo"
if _REPO not in sys.path:
    sys.path.insert(0, _REPO)

D = 1024       # d_model
S = 1024       # sequence length
B = 4          # batch
H = 16         # heads
DK = 64        # head dim
HALF = 512     # query rows per core
N_CORES = 8
LN_EPS = 1e-5
LN16 = float(np.log(16.0))

USE_PB = True   # gpsimd partition_broadcast for the 1/denom row
USE_DR = True   # fp8 DoubleRow for the V and O projections
import os as _os
DEBUG_DUMP = _os.environ.get("KDEBUG", "0") == "1"

_cache = {}


def _build_nc(ln_trivial=True, use_pb=USE_PB, use_dr=USE_DR):
    import concourse.bacc as bacc
    import concourse.mybir as mybir
    import concourse.tile as tile
    import concourse.bass as bass

    f32 = mybir.dt.float32
    bf16 = mybir.dt.bfloat16
    fp8 = mybir.dt.float8e4
    ADD = mybir.AluOpType.add
    MULT = mybir.AluOpType.mult
    DRM = mybir.MatmulPerfMode.DoubleRow
    EXPF = mybir.ActivationFunctionType.Exp

    wdt = fp8 if use_dr else bf16

    nc = bacc.Bacc(None, target_bir_lowering=False)

    # ---- DRAM parameters (per-core views, SPMD-identical program) ----
    # All inputs host-prepacked to [128, ...] contiguous layouts.
    xt8_p = nc.declare_dram_parameter("xt8", [128, 8, S], wdt, isOutput=False)    # x^T (rotated), V-proj dtype
    xtb_p = nc.declare_dram_parameter("xtb", [128, 8 * S], bf16, isOutput=False)  # x^T (rotated), bf16
    wq_p = nc.declare_dram_parameter("wq", [128, 8 * D], bf16, isOutput=False)    # (q_w^eff)/8
    wk_p = nc.declare_dram_parameter("wk", [128, 8 * D], bf16, isOutput=False)    # k_w^eff
    wv_p = nc.declare_dram_parameter("wv8", [128, 8 * D], wdt, isOutput=False)    # 16*v_w^eff (DR-packed)
    wo_p = nc.declare_dram_parameter("wo8", [128, 8 * D], wdt, isOutput=False)    # 16*o_w^eff (DR-packed)
    bq_p = nc.declare_dram_parameter("bqa", [128, 8], f32, isOutput=False)        # q_b/8 arranged [p, ot]
    xr_p = nc.declare_dram_parameter("xr", [128, 4 * D], bf16, isOutput=False)    # 16*(x rows + o_b')
    if not ln_trivial:
        lng_p = nc.declare_dram_parameter("lng", [D], f32, isOutput=False)
        lnb_p = nc.declare_dram_parameter("lnb", [D], f32, isOutput=False)
    normed_p = nc.declare_dram_parameter("normed", [HALF, D], bf16, isOutput=True)
    attn_ps = [nc.declare_dram_parameter(f"attn{pr}", [S, HALF], bf16, isOutput=True)
               for pr in range(8)]
    if DEBUG_DUMP:
        dbg_qt = nc.declare_dram_parameter("dbg_qt", [128, 8 * HALF], bf16, isOutput=True)
        dbg_kt = nc.declare_dram_parameter("dbg_kt", [128, 8 * S], bf16, isOutput=True)
        dbg_exp = nc.declare_dram_parameter("dbg_exp", [128, 8 * 2 * HALF], bf16, isOutput=True)
        dbg_vg = nc.declare_dram_parameter("dbg_vg", [128, 8 * 16 * 66], bf16, isOutput=True)
        dbg_ctx = nc.declare_dram_parameter("dbg_ctx", [128, 8 * HALF], wdt, isOutput=True)
        dbg_rbc = nc.declare_dram_parameter("dbg_rbc", [128, HALF], bf16, isOutput=True)

    def bcast_ap(handle, dims):
        ap = handle.ap()
        return bass.AP(tensor=ap.tensor, offset=ap.offset, ap=[[0, 128]] + dims)

    with tile.TileContext(nc) as tc:
        with (
            tc.tile_pool(name="consts", bufs=1) as consts,
            tc.tile_pool(name="res", bufs=1) as res,
            tc.tile_pool(name="exp", bufs=2) as epool,
            tc.tile_pool(name="asum", bufs=3) as aspool,
            tc.tile_pool(name="tmpw", bufs=1) as napool,
            tc.tile_pool(name="rcp", bufs=2) as rpool,
            tc.tile_pool(name="psA", bufs=2, space="PSUM") as psA,
            tc.tile_pool(name="psS", bufs=2, space="PSUM") as psS,
            tc.tile_pool(name="psC", bufs=2, space="PSUM") as psC,
        ):
            # ---------------- streaming inputs (freed before the LN tail) --
            stream_cm = tc.tile_pool(name="stream", bufs=1)
            stream = stream_cm.__enter__()

            BQ = consts.tile([128, 8], f32, tag="bq")
            nc.sync.dma_start(out=BQ, in_=bq_p[:, :])
            # token-half split so V-proj (tt 0-3) starts as early as possible
            XT8 = stream.tile([128, 8, S], wdt, tag="XT8")
            nc.sync.dma_start(out=XT8[:, :, 0:HALF], in_=xt8_p[:, :, 0:HALF])
            WV8 = stream.tile([128, 4, 2, D] if use_dr else [128, 8, D], wdt, tag="WV8")
            nc.sync.dma_start(out=WV8, in_=wv_p[:, :])
            nc.sync.dma_start(out=XT8[:, :, HALF:S], in_=xt8_p[:, :, HALF:S])
            XTB = stream.tile([128, 8, S], bf16, tag="XTB")
            nc.sync.dma_start(out=XTB, in_=xtb_p[:, :])
            WK = stream.tile([128, 8, D], bf16, tag="WK")
            nc.sync.dma_start(out=WK, in_=wk_p[:, :])
            WQ = stream.tile([128, 8, D], bf16, tag="WQ")
            nc.sync.dma_start(out=WQ, in_=wq_p[:, :])
            WO8 = consts.tile([128, 4, 2, D] if use_dr else [128, 8, D], wdt, tag="WO8")
            nc.sync.dma_start(out=WO8, in_=wo_p[:, :])
            XR = consts.tile([128, 4, D], bf16, tag="XR")
            nc.sync.dma_start(out=XR, in_=xr_p[:, :])
            if not ln_trivial:
                GLN = consts.tile([128, D], f32, tag="gln")
                nc.sync.dma_start(out=GLN, in_=bcast_ap(lng_p, [[1, D]]))
                BLN = consts.tile([128, D], f32, tag="bln")
                nc.sync.dma_start(out=BLN, in_=bcast_ap(lnb_p, [[1, D]]))
            EPS = consts.tile([128, 1], f32, tag="eps")
            nc.vector.memset(EPS, LN_EPS * 256.0)
            BEXP = consts.tile([128, 1], f32, tag="bexp")
            nc.vector.memset(BEXP, LN16)
            if not use_pb:
                ONES = consts.tile([1, 128], bf16, tag="ones")
                nc.vector.memset(ONES, 1.0)

            # ---------------- persistent activations ----------------
            KT = res.tile([128, 8, S], bf16, tag="KT")          # [dk%128, pair, k-token]
            QT = res.tile([128, 8, HALF], bf16, tag="QT")       # [dk%128, pair, q]
            VG = res.tile([128, 8, 16, 66], bf16, tag="VG")     # [k%128, kt, head, dk+ones(+pad)]
            CTX = res.tile([128, 8, HALF], wdt, tag="CTX")      # [d%128, pair, token]

            nc.vector.memset(VG[:, :, :, 64:65], 1.0)

            # ======== V projection (fp8 DoubleRow) ========
            for tt in range(8):
                for ncr in range(2):
                    ps = psA.tile([128, HALF], f32, tag="acc")
                    if use_dr:
                        for i in range(4):
                            nc.tensor.matmul(
                                ps,
                                lhsT=XT8[:, 2 * i:2 * i + 2, tt * 128:(tt + 1) * 128],
                                rhs=WV8[:, i, :, ncr * 512:(ncr + 1) * 512],
                                start=(i == 0), stop=(i == 3),
                                perf_mode=DRM,
                            )
                    else:
                        for it in range(8):
                            nc.tensor.matmul(
                                ps,
                                lhsT=XT8[:, it, tt * 128:(tt + 1) * 128],
                                rhs=WV8[:, it, ncr * 512:(ncr + 1) * 512],
                                start=(it == 0), stop=(it == 7),
                            )
                    ps3 = ps.rearrange("p (h c) -> p h c", c=64)
                    nc.scalar.copy(VG[:, tt, ncr * 8:(ncr + 1) * 8, 0:64], ps3)

            # ======== software-pipelined pair loop ========
            # iteration i emits: K/Q projection + scores/exp for pair i,
            # and PV + normalize + mean for pair i-1.
            def emit_kq(pr):
                for ncr in range(2):
                    ps = psA.tile([128, HALF], f32, tag="acc")
                    for it in range(8):
                        nc.tensor.matmul(
                            ps,
                            lhsT=WK[:, it, pr * 128:(pr + 1) * 128],
                            rhs=XTB[:, it, ncr * 512:(ncr + 1) * 512],
                            start=(it == 0), stop=(it == 7),
                        )
                    nc.vector.tensor_copy(KT[:, pr, ncr * 512:(ncr + 1) * 512], ps)
                ps = psA.tile([128, HALF], f32, tag="acc")
                for it in range(8):
                    nc.tensor.matmul(
                        ps,
                        lhsT=WQ[:, it, pr * 128:(pr + 1) * 128],
                        rhs=XTB[:, it, 0:HALF],
                        start=(it == 0), stop=(it == 7),
                    )
                nc.scalar.add(QT[:, pr, :], ps, BQ[:, pr:pr + 1])

            def emit_scores(pr):
                # scores^T ([k, q]) for both heads of the pair; the two 64-row
                # lhsT slices occupy disjoint PE row groups -> run concurrently
                EXPp = epool.tile([128, 8, 2, HALF], bf16, tag="exp", name=f"EXP{pr}")
                for kt in range(8):
                    sp = psS.tile([128, 2, HALF], f32, tag="sps")
                    for hh in range(2):
                        nc.tensor.matmul(
                            sp[:, hh, :],
                            lhsT=KT[hh * 64:hh * 64 + 64, pr, kt * 128:(kt + 1) * 128],
                            rhs=QT[hh * 64:hh * 64 + 64, pr, :],
                            start=True, stop=True,
                        )
                    # E = 16*exp(s): the 16 cancels against rbc = 1/(256*denom)
                    nc.scalar.activation(EXPp[:, kt, :, :], sp, EXPF, bias=BEXP[:, 0:1])
                return EXPp

            def emit_pv(pr, EXPp):
                ASUM = aspool.tile([128, 8, HALF], bf16, tag="asum", name=f"ASUM{pr}")
                for hh in range(2):
                    h = 2 * pr + hh
                    cp = psC.tile([65, HALF], f32, tag="cps")
                    for kt in range(8):
                        nc.tensor.matmul(
                            cp,
                            lhsT=VG[:, kt, h, 0:65],
                            rhs=EXPp[:, kt, hh, :],
                            start=(kt == 0), stop=(kt == 7),
                        )
                    # rbc = 1/(256*denom) broadcast to all partitions
                    # (reciprocal_approx_fast needs an SBUF source on HW)
                    dcp = rpool.tile([1, HALF], f32, tag="dcp")
                    nc.scalar.copy(dcp, cp[64:65, :])
                    rec = rpool.tile([1, HALF], f32, tag="rec")
                    nc.vector.reciprocal_approx_fast(out=rec, in_=dcp)
                    rec_bf = rpool.tile([1, HALF], bf16, tag="recbf")
                    nc.vector.tensor_scalar_mul(rec_bf, rec, 1.0 / 16.0)
                    rbc = rpool.tile([128, HALF], bf16, tag="rbc")
                    if use_pb:
                        nc.gpsimd.partition_broadcast(rbc[:, :], rec_bf[:, :])
                    else:
                        db = psA.tile([128, HALF], f32, tag="acc")
                        nc.tensor.matmul(db, lhsT=ONES, rhs=rec_bf, start=True, stop=True)
                        nc.scalar.copy(rbc, db)
                    if DEBUG_DUMP and pr == 0 and hh == 0:
                        nc.sync.dma_start(out=dbg_rbc[:, :], in_=rbc[:, :])

                    # normalized ctx (true scale) into persistent CTX
                    nc.vector.tensor_tensor(
                        CTX[hh * 64:hh * 64 + 64, pr, :], cp[0:64, :], rbc[0:64, :], MULT,
                    )

                    # attention-mean accumulation: sum over the pair's 2 heads
                    rbc_ap = rbc[:, :]
                    rbc_w = bass.AP(tensor=rbc_ap.tensor, offset=rbc_ap.offset,
                                    ap=[rbc_ap.ap[0], [0, 8], rbc_ap.ap[1]])
                    if hh == 0:
                        nc.vector.tensor_tensor(ASUM[:, :, :], EXPp[:, :, 0, :], rbc_w, MULT)
                    else:
                        tmpw = napool.tile([128, 8, HALF], bf16, tag="nrmattn")
                        nc.vector.tensor_tensor(tmpw, EXPp[:, :, 1, :], rbc_w, MULT)
                        nc.vector.tensor_tensor(ASUM[:, :, :], tmpw, ASUM[:, :, :], ADD)
                nc.sync.dma_start(
                    out=bass.AP(
                        tensor=attn_ps[pr].ap().tensor, offset=0,
                        ap=[[HALF, 128], [128 * HALF, 8], [1, HALF]],
                    ),
                    in_=ASUM[:, :, :],
                )

            prev = None
            for pr in range(8):
                emit_kq(pr)
                if prev is not None:
                    emit_pv(pr - 1, prev)
                prev = emit_scores(pr)
                if DEBUG_DUMP and pr == 0:
                    nc.sync.dma_start(out=dbg_exp[:, :], in_=prev[:, :, :, :])
            emit_pv(7, prev)

            if DEBUG_DUMP:
                nc.sync.dma_start(out=dbg_qt[:, :], in_=QT[:, :, :])
                nc.sync.dma_start(out=dbg_kt[:, :], in_=KT[:, :, :])
                nc.sync.dma_start(out=dbg_vg[:, :], in_=VG[:, :, :, :])
                nc.sync.dma_start(out=dbg_ctx[:, :], in_=CTX[:, :, :])

            stream_cm.__exit__(None, None, None)

            # ======== output projection + residual + LayerNorm ========
            with tc.tile_pool(name="ln", bufs=2) as lpool:
                for tt in range(4):
                    hh_t = lpool.tile([128, D], bf16, tag="hh")
                    for ncr in range(2):
                        ps = psA.tile([128, HALF], f32, tag="acc")
                        if use_dr:
                            for i in range(4):
                                nc.tensor.matmul(
                                    ps,
                                    lhsT=CTX[:, 2 * i:2 * i + 2, tt * 128:(tt + 1) * 128],
                                    rhs=WO8[:, i, :, ncr * 512:(ncr + 1) * 512],
                                    start=(i == 0), stop=(i == 3),
                                    perf_mode=DRM,
                                )
                        else:
                            for it in range(8):
                                nc.tensor.matmul(
                                    ps,
                                    lhsT=CTX[:, it, tt * 128:(tt + 1) * 128],
                                    rhs=WO8[:, it, ncr * 512:(ncr + 1) * 512],
                                    start=(it == 0), stop=(it == 7),
                                )
                        # hh = 16*out + 16*(x + o_b') = 16*h
                        nc.vector.tensor_tensor(
                            hh_t[:, ncr * 512:(ncr + 1) * 512], ps,
                            XR[:, tt, ncr * 512:(ncr + 1) * 512], ADD)

                    st = lpool.tile([128, 2, 6], f32, tag="st")
                    for g2 in range(2):
                        nc.vector.bn_stats(st[:, g2, :], hh_t[:, g2 * 512:(g2 + 1) * 512])
                    mv = lpool.tile([128, 2], f32, tag="mv")
                    nc.vector.bn_aggr(mv, st)
                    nmu = lpool.tile([128, 1], f32, tag="nmu")
                    nc.vector.tensor_scalar_mul(nmu, mv[:, 0:1], -1.0)
                    sd = lpool.tile([128, 1], f32, tag="sd")
                    nc.scalar.activation(
                        sd, mv[:, 1:2], mybir.ActivationFunctionType.Sqrt,
                        bias=EPS[:, 0:1], scale=1.0,
                    )
                    rstd = lpool.tile([128, 1], f32, tag="rstd")
                    nc.vector.reciprocal(rstd, sd)

                    t1 = lpool.tile([128, D], bf16, tag="t1")
                    nc.vector.tensor_scalar(t1, hh_t, nmu[:, 0:1], rstd[:, 0:1], ADD, MULT)
                    if not ln_trivial:
                        t2 = lpool.tile([128, D], f32, tag="t2")
                        nc.vector.tensor_tensor(t2, t1, GLN, MULT)
                        nrm = lpool.tile([128, D], bf16, tag="nrm")
                        nc.vector.tensor_tensor(nrm, t2, BLN, ADD)
                        t1 = nrm
                    nc.sync.dma_start(out=normed_p[tt * 128:(tt + 1) * 128, :], in_=t1)

    nc.finalize()
    return nc


def _get_nc(ln_trivial=True):
    key = f"nc_{ln_trivial}_{USE_PB}_{USE_DR}"
    if key not in _cache:
        _cache[key] = _build_nc(ln_trivial=ln_trivial)
    return _cache[key]


def _pack128(m):
    # [8*128, F] -> [128, 8, F]: out[p, i, f] = m[i*128 + p, f]
    F = m.shape[1]
    return np.ascontiguousarray(m.reshape(8, 128, F).transpose(1, 0, 2)).reshape(128, 8 * F)


def _pack_dr(m):
    # DoubleRow pack: [8*128, F] -> [128, 4, 2, F]: out[p, i, j, f] = m[(2i+j)*128+p, f]
    F = m.shape[1]
    return np.ascontiguousarray(m.reshape(4, 2, 128, F).transpose(2, 0, 1, 3)).reshape(128, 8 * F)


def _prep_in_maps(inputs):
    import ml_dtypes
    bf = ml_dtypes.bfloat16
    f8 = ml_dtypes.float8_e4m3fn

    x = np.asarray(inputs["x"], dtype=np.float32)
    w_eff = {}
    for p in ("q", "k", "v", "o"):
        w = np.asarray(inputs[f"{p}_w"], dtype=np.float32)
        A = np.asarray(inputs[f"{p}_A"], dtype=np.float32)
        Bm = np.asarray(inputs[f"{p}_B"], dtype=np.float32)
        w_eff[p] = w.T + 2.0 * (A @ Bm)          # [in, out]

    def to8(m):
        return np.clip(m, -240.0, 240.0).astype(f8)

    wq = _pack128(w_eff["q"] / 8.0).astype(bf)
    wk = _pack128(w_eff["k"]).astype(bf)
    if USE_DR:
        wv = _pack_dr(to8(16.0 * w_eff["v"]))
        wo = _pack_dr(to8(16.0 * w_eff["o"]))
    else:
        wv = _pack128((16.0 * w_eff["v"])).astype(bf)
        wo = _pack128((16.0 * w_eff["o"])).astype(bf)
    bqa = np.ascontiguousarray(
        (np.asarray(inputs["q_b"], np.float32) / 8.0).reshape(8, 128).T)
    # v_b folds into the output bias exactly (attention probs sum to 1)
    ob_eff = np.asarray(inputs["o_b"], np.float32) \
        + np.asarray(inputs["v_b"], np.float32) @ w_eff["o"]
    lng = np.ascontiguousarray(inputs["ln_g"], dtype=np.float32)
    lnb = np.ascontiguousarray(inputs["ln_b"], dtype=np.float32)

    in_maps = []
    for c in range(N_CORES):
        b, qh = c // 2, c % 2
        xb = x[b]                                  # [S, D]
        xT = np.ascontiguousarray(xb.T)            # [D, S]
        if qh == 1:
            xT = np.concatenate([xT[:, HALF:], xT[:, :HALF]], axis=1)
        xt8 = (_pack128(xT).astype(f8) if USE_DR
               else _pack128(xT).astype(bf)).reshape(128, 8, S)
        xtb = _pack128(xT).astype(bf)
        rows = 16.0 * (xb[qh * HALF:(qh + 1) * HALF, :] + ob_eff[None, :])
        xr = np.ascontiguousarray(
            rows.reshape(4, 128, D).transpose(1, 0, 2)).reshape(128, 4 * D).astype(bf)
        in_maps.append({
            "xt8": xt8, "xtb": xtb, "xr": xr,
            "wq": wq, "wk": wk, "wv8": wv, "wo8": wo,
            "bqa": bqa,
            **({} if _ln_trivial(inputs) else {"lng": lng, "lnb": lnb}),
        })
    return in_maps


def _ln_trivial(inputs):
    return bool(
        np.all(np.asarray(inputs["ln_g"]) == 1.0)
        and np.all(np.asarray(inputs["ln_b"]) == 0.0))


def run_on_device(inputs, trace=False, tmpdir=None):
    from concourse.bass_utils import run_bass_kernel_spmd

    nc = _get_nc(ln_trivial=_ln_trivial(inputs))
    in_maps = _prep_in_maps(inputs)
    res = run_bass_kernel_spmd(
        nc, in_maps, core_ids=list(range(N_CORES)), trace=trace, tmpdir=tmpdir,
    )

    normed = np.zeros((B, S, D), dtype=np.float32)
    attn_mean = np.zeros((B, S, S), dtype=np.float32)
    for c in range(N_CORES):
        b, qh = c // 2, c % 2
        normed[b, qh * HALF:(qh + 1) * HALF, :] = np.asarray(
            res.results[c]["normed"], dtype=np.float32)
    for b in range(B):
        halves = []
        for qh in range(2):
            r = res.results[2 * b + qh]
            A = np.zeros((S, HALF), dtype=np.float32)
            for pr in range(8):
                A += np.asarray(r[f"attn{pr}"], dtype=np.float32)
            if qh == 1:
                A = np.concatenate([A[HALF:], A[:HALF]], axis=0)  # undo k-perm
            halves.append(A)                       # [S(k), HALF(q)]
        attn_mean[b] = np.concatenate(halves, axis=1).T
    return (normed, attn_mean), res


def kernel(**inputs):
    (normed, attn_mean), _ = run_on_device(inputs, trace=False)
    return normed, attn_mean
